# revision 1
# baseline (speedup 1.0000x reference)
"""PVT-style spatial-reduction attention on 8 TRN2 NeuronCores.

Sharding: data-parallel over batch (B=8 -> 1 image per core), no collectives.

Per-core math (C=64, N=16384=128x128, HEADS=2, dh=32, SR=8 -> Nk=256):
  q  = x @ (Wq*scale)                      [N, 64]
  cv = conv8x8s8(x) + b_sr                 [256, 64]   (as 64 accumulating MMs)
  z  = LayerNorm_C(cv) ; kv = z @ (g*Wkv)  [256, 128] -> k,v per head [256,32]
  ST = k @ qT  (scores transposed, keys on partitions)
  E  = exp(ST)            (no max-shift: |scores| < ~0.3 by construction)
  av = [v|1]^T @ E        -> numerator rows + denominator row per head
  y  = (num/den) @ Wproj + b_proj
All matmuls bf16 inputs / f32 PSUM. Host does layout only (transpose, cast,
weight folding); all FLOPs on device.
"""

import numpy as np
import ml_dtypes

import concourse.bass as bass
import concourse.bacc as bacc
import concourse.tile as tile
from concourse import mybir
from concourse.bass_utils import run_bass_kernel_spmd

BF16 = mybir.dt.bfloat16
F32 = mybir.dt.float32

B, N, C = 8, 16384, 64
H, DH, SR, NK = 2, 32, 8, 256
EPS = 1e-5
NCORES = 8

_CACHE = {}


def _bf16(a):
    return np.asarray(a, dtype=ml_dtypes.bfloat16)


def build_graph():
    nc = bacc.Bacc("TRN2", target_bir_lowering=False)

    xt_d = nc.declare_dram_parameter("xt", [C, N], BF16, isOutput=False)
    packb_d = nc.declare_dram_parameter("packb", [128, 5120], BF16,
                                        isOutput=False)
    out_d = nc.declare_dram_parameter("out", [N, C], F32, isOutput=True)

    with tile.TileContext(nc) as tc:
        with (
            tc.tile_pool(name="persist", bufs=1) as pp,
            tc.tile_pool(name="work", bufs=3) as wp,
        ):
            # ---- load persistent tensors (3 DMAs total) ----
            xt_sb = pp.tile([C, N], BF16, tag="xt")
            nc.sync.dma_start(out=xt_sb[:], in_=xt_d[:])
            packb_sb = pp.tile([128, 5120], BF16, tag="packb")
            nc.sync.dma_start(out=packb_sb[:], in_=packb_d[:])
            wqd_sb = packb_sb[0:64, 0:128]
            wsr_sb = packb_sb[0:64, 128:4224].rearrange(
                "c (p o) -> c p o", p=64)
            wkvg_sb = packb_sb[0:64, 4224:4352]
            ncs_sb = packb_sb[0:1, 4352:4480]
            inv64_sb = packb_sb[0:64, 4480:4481]
            ident_sb = packb_sb[:, 4481:4513]
            wprojA_sb = packb_sb[0:32, 4513:4577]
            wprojB_sb = packb_sb[0:32, 5025:5089]
            wprojAB = [wprojA_sb, wprojB_sb]
            one32_sb = packb_sb[32:33, 5089:5090]
            bsrrow_sb = packb_sb[0:1, 4577:4641]
            ckvrow_sb = packb_sb[0:1, 4641:4769]
            onesrow_sb = packb_sb[0:1, 4769:5025]

            qt_sb = pp.tile([128, N], BF16, tag="qt")
            kvt_sb = pp.tile([128, NK], BF16, tag="kvt")
            krep_sb = pp.tile([128, NK], BF16, tag="krep")
            vext = [
                [pp.tile([128, 64], BF16, tag=f"vext{h}{c}",
                         name=f"vext{h}{c}") for c in range(2)]
                for h in range(2)
            ]

            # ================= prelude: conv + LN stats + kv =================
            with tc.tile_pool(name="pre_ps", bufs=1, space="PSUM") as pps:
                conv_ps = pps.tile([C, NK], F32, tag="conv")
                xv = xt_sb[:].rearrange("c (ph i pw j) -> c ph i pw j",
                                        ph=16, i=8, pw=16, j=8)
                for pos in range(64):
                    i, j = pos // 8, pos % 8
                    nc.tensor.matmul(
                        conv_ps[:], wsr_sb[:, pos, :], xv[:, :, i, :, j],
                        start=(pos == 0), stop=False,
                    )
                nc.tensor.matmul(conv_ps[:], bsrrow_sb, onesrow_sb,
                                 start=False, stop=True)
                c_sb = pp.tile([C, NK], BF16, tag="c_sb")
                nc.vector.tensor_copy(c_sb[:], conv_ps[:])
                sq_sb = pp.tile([C, NK], BF16, tag="sq_sb")
                nc.scalar.square(sq_sb[:], c_sb[:])

                mu_ps = pps.tile([1, NK], F32, tag="mu")
                nc.tensor.matmul(mu_ps[:], inv64_sb, c_sb[:],
                                 start=True, stop=True)
                m2_ps = pps.tile([1, NK], F32, tag="m2")
                nc.tensor.matmul(m2_ps[:], inv64_sb, sq_sb[:],
                                 start=True, stop=True)

                mu16_sb = pp.tile([1, NK], BF16, tag="mu16")
                nc.vector.tensor_copy(mu16_sb[:], mu_ps[:])
                mu2_sb = wp.tile([1, NK], F32, tag="mu2")
                nc.vector.tensor_tensor(mu2_sb[:], mu16_sb[:], mu16_sb[:],
                                        mybir.AluOpType.mult)
                var_sb = wp.tile([1, NK], F32, tag="var")
                nc.vector.tensor_tensor(var_sb[:], m2_ps[:], mu2_sb[:],
                                        mybir.AluOpType.subtract)
                veps_sb = wp.tile([1, NK], F32, tag="veps")
                nc.vector.tensor_scalar_add(veps_sb[:], var_sb[:], EPS)
                rvar_sb = wp.tile([1, NK], F32, tag="rvar")
                nc.vector.reciprocal(rvar_sb[:], veps_sb[:])
                rstd_sb = pp.tile([1, NK], F32, tag="rstd")
                nc.scalar.sqrt(rstd_sb[:], rvar_sb[:])

                # kvT = WkvgT @ (conv+bsr) - colsum x mu, then *rstd + ckv
                kv_ps = pps.tile([128, NK], F32, tag="kv")
                nc.tensor.matmul(kv_ps[:], wkvg_sb, c_sb[:],
                                 start=True, stop=False)
                nc.tensor.matmul(kv_ps[:], ncs_sb, mu16_sb[:],
                                 start=False, stop=False)
                nc.tensor.matmul(kv_ps[:], ckvrow_sb, onesrow_sb,
                                 start=False, stop=True)
                rstd_rep = pp.tile([128, NK], F32, tag="rstd_rep",
                                   name="rstd_rep")
                nc.gpsimd.partition_broadcast(rstd_rep[:], rstd_sb[:])
                nc.vector.tensor_tensor(
                    kvt_sb[:], kv_ps[:], rstd_rep[:],
                    mybir.AluOpType.mult)

                # krep: kT_h0 at rows 0-31, kT_h1 at rows 64-95
                nc.vector.tensor_copy(krep_sb[0:32, :], kvt_sb[0:32, :])
                nc.sync.dma_start(out=krep_sb[64:96, :], in_=kvt_sb[32:64, :])

                # v natural layout per (head, key-chunk) + ones column
                for h in range(2):
                    for ch in range(2):
                        tp_ps = pps.tile([128, 32], BF16, tag="tp")
                        nc.tensor.matmul(
                            tp_ps[:],
                            kvt_sb[64 + 32 * h:96 + 32 * h,
                                   128 * ch:128 * (ch + 1)],
                            ident_sb[64 + 32 * h:96 + 32 * h, :],
                            is_transpose=True,
                            tile_position=(64 + 32 * h, 0),
                            start=True, stop=True)
                        nc.vector.memset(vext[h][ch][:, 32:64], 0.0)
                        nc.vector.memset(vext[h][ch][:, 32:33], 1.0)
                        nc.vector.tensor_copy(vext[h][ch][:, 0:32], tp_ps[:])

                # ---- qT (dup layout rows = [qh0, qh1, qh0, qh1]) ----
                for t in range(32):
                    qt_ps = pps.tile([128, 512], F32, tag="qtp")
                    nc.tensor.matmul(
                        qt_ps[:], wqd_sb, xt_sb[:, 512 * t:512 * (t + 1)],
                        start=True, stop=True)
                    nc.vector.tensor_copy(
                        qt_sb[:, 512 * t:512 * (t + 1)], qt_ps[:])

            # ================= attention main loop =================
            TCH = 256          # tokens per chunk
            NCH = N // TCH     # 64 chunks
            GRP = 4            # chunks per epilogue group
            with (
                tc.tile_pool(name="st_ps", bufs=2, space="PSUM") as stp,
                tc.tile_pool(name="av_ps", bufs=1, space="PSUM") as avp,
                tc.tile_pool(name="y_ps", bufs=2, space="PSUM") as yp,
                tc.tile_pool(name="rd_ps", bufs=1, space="PSUM") as rdp,
                tc.tile_pool(name="att_sb", bufs=4) as asb,
                tc.tile_pool(name="grp_sb", bufs=2) as gsb,
            ):
                for g in range(NCH // GRP):
                    # av_sb: [64, chunk, 512] ; per chunk h0 toks at
                    # [0:256], h1 at [256:512]; row 32 = denominator
                    av_sb = gsb.tile([C, GRP, 512], BF16, tag="av_sb")
                    for cc in range(GRP):
                        t = g * GRP + cc
                        st_ps = stp.tile([128, 4 * TCH], F32, tag="st")
                        for hh in range(2):
                            for kc in range(2):
                                qr = 2 * hh + kc
                                rg = 64 * hh
                                nc.tensor.matmul(
                                    st_ps[:, TCH * qr:TCH * (qr + 1)],
                                    krep_sb[rg:rg + 32,
                                            128 * kc:128 * (kc + 1)],
                                    qt_sb[rg:rg + 32,
                                          TCH * t:TCH * (t + 1)],
                                    start=True, stop=True,
                                    tile_position=(rg, 0),
                                    skip_group_check=True,
                                )
                        est_sb = asb.tile([128, 4 * TCH], BF16, tag="est")
                        nc.scalar.activation(est_sb[:], st_ps[:],
                                             mybir.ActivationFunctionType.Exp)
                        av_ps = avp.tile([C, 512], F32, tag="av")
                        for hh in range(2):
                            for kc in range(2):
                                nc.tensor.matmul(
                                    av_ps[:, 256 * hh:256 * (hh + 1)],
                                    vext[hh][kc][:],
                                    est_sb[:, TCH * (2 * hh + kc):
                                           TCH * (2 * hh + kc + 1)],
                                    start=(kc == 0), stop=(kc == 1),
                                    skip_group_check=True,
                                )
                        nc.vector.tensor_copy(av_sb[:, cc, :], av_ps[:])

                    # denominator rows -> columns via K=1 matmuls
                    rd_ps = rdp.tile([128, 16], F32, tag="rd")
                    for sl in range(2 * GRP):
                        chunk, wi = sl // 2, sl % 2
                        for hh in range(2):
                            nc.tensor.matmul(
                                rd_ps[:, 8 * hh + sl:8 * hh + sl + 1],
                                av_sb[32:33, chunk,
                                      256 * hh + 128 * wi:
                                      256 * hh + 128 * wi + 128],
                                one32_sb,
                                start=True, stop=True,
                                tile_position=(32, 0),
                                skip_group_check=True,
                            )
                    rdinv_sb = gsb.tile([128, 16], F32, tag="rdinv")
                    nc.vector.reciprocal(rdinv_sb[:], rd_ps[:])

                    y_sb = gsb.tile([128, GRP * 2, C], F32, tag="y_sb")
                    for sl in range(GRP * 2):
                        chunk, wi = sl // 2, sl % 2
                        y2_ps = yp.tile([128, 2 * C], F32, tag="y")
                        for hh in range(2):
                            nc.tensor.matmul(
                                y2_ps[:, C * hh:C * (hh + 1)],
                                av_sb[0:32, chunk,
                                      256 * hh + 128 * wi:
                                      256 * hh + 128 * wi + 128],
                                wprojAB[hh],
                                start=True, stop=True,
                                skip_group_check=True,
                            )
                        tA_sb = wp.tile([128, C], F32, tag="tA")
                        nc.vector.tensor_scalar_mul(
                            tA_sb[:], y2_ps[:, 0:C],
                            rdinv_sb[:, sl:sl + 1])
                        tB_sb = wp.tile([128, C], F32, tag="tB")
                        nc.vector.tensor_scalar_mul(
                            tB_sb[:], y2_ps[:, C:2 * C],
                            rdinv_sb[:, 8 + sl:8 + sl + 1])
                        nc.vector.tensor_tensor(
                            y_sb[:, sl, :], tA_sb[:], tB_sb[:],
                            mybir.AluOpType.add)
                    ov = out_d[:].rearrange("(g s p) c -> g p s c",
                                            g=NCH // GRP, s=GRP * 2, p=128)
                    nc.sync.dma_start(out=ov[g], in_=y_sb[:])

    nc.compile()
    return nc


def _prep_inputs(x, height, width, Wq, Wkv, Wsr, b_sr, ln_g, ln_b,
                 Wproj, b_proj):
    x = np.asarray(x, np.float32)
    Wq = np.asarray(Wq, np.float32)
    Wkv = np.asarray(Wkv, np.float32)
    Wsr = np.asarray(Wsr, np.float32)
    b_sr = np.asarray(b_sr, np.float32)
    ln_g = np.asarray(ln_g, np.float32)
    ln_b = np.asarray(ln_b, np.float32)
    Wproj = np.asarray(Wproj, np.float32)

    scale = float(DH) ** -0.5
    wq_s = Wq * scale
    wqd = np.zeros((C, 128), np.float32)
    wqd[:, 0:32] = wq_s[:, 0:32]    # head0 -> psum rows 0-31
    wqd[:, 64:96] = wq_s[:, 32:64]  # head1 -> psum rows 64-95
    wsr = Wsr.transpose(1, 2, 3, 0).reshape(C, 64 * C)          # [ci,pos*co]
    wkvg = ln_g[:, None] * Wkv                                  # [64,128]
    ncs = (-wkvg.sum(axis=0)).reshape(1, 128)
    ckv = (ln_b @ Wkv).reshape(128, 1).astype(np.float32)
    packb = np.zeros((128, 5120), np.float32)
    packb[0:64, 0:128] = wqd
    packb[0:64, 128:4224] = wsr
    packb[0:64, 4224:4352] = wkvg
    packb[0:1, 4352:4480] = ncs
    packb[0:64, 4480] = 1.0 / C
    packb[:, 4481:4513] = np.tile(np.eye(32, dtype=np.float32), (4, 1))
    packb[0:32, 4513:4577] = Wproj[0:32]
    packb[0:32, 5025:5089] = Wproj[32:64]
    packb[0, 4577:4641] = b_sr
    packb[0, 4641:4769] = ckv[:, 0]
    packb[0, 4769:5025] = 1.0
    packb[32, 5089] = 1.0
    packb = _bf16(packb)
    shared = dict(packb=packb)
    in_maps = []
    for b in range(B):
        m = dict(shared)
        m["xt"] = _bf16(np.ascontiguousarray(x[b].T))
        in_maps.append(m)
    return in_maps


def kernel(x, height, width, Wq, Wkv, Wsr, b_sr, ln_g, ln_b, Wproj, b_proj,
           _want_time=False):
    assert int(height) == 128 and int(width) == 128
    in_maps = _prep_inputs(x, height, width, Wq, Wkv, Wsr, b_sr, ln_g, ln_b,
                           Wproj, b_proj)
    if "nc" not in _CACHE:
        _CACHE["nc"] = build_graph()
    nc = _CACHE["nc"]
    import os
    trace = bool(int(os.environ.get("BASS_KERNEL_TRACE", "0")))
    res = run_bass_kernel_spmd(nc, in_maps, core_ids=list(range(NCORES)),
                               trace=trace)
    outs = [np.asarray(res.results[i]["out"], np.float32) for i in range(B)]
    out = np.stack(outs, axis=0)
    out = out + np.asarray(b_proj, np.float32)[None, None, :]
    if _want_time:
        return out, res
    return out



# revision 3
# speedup vs baseline: 8.8837x; 8.8837x over previous
"""PVT-style spatial-reduction attention on 8 TRN2 NeuronCores.

Sharding: data-parallel over batch (B=8 -> 1 image per core), no collectives.

Per-core math (C=64, N=16384=128x128, HEADS=2, dh=32, SR=8 -> Nk=256):
  q  = x @ (Wq*scale)                      [N, 64]
  cv = conv8x8s8(x) + b_sr                 [256, 64]   (as 64 accumulating MMs)
  z  = LayerNorm_C(cv) ; kv = z @ (g*Wkv)  [256, 128] -> k,v per head [256,32]
  ST = k @ qT  (scores transposed, keys on partitions)
  E  = exp(ST)            (no max-shift: |scores| < ~0.3 by construction)
  av = [v|1]^T @ E        -> numerator rows + denominator row per head
  y  = (num/den) @ Wproj + b_proj
All matmuls bf16 inputs / f32 PSUM. Host does layout only (transpose, cast,
weight folding); all FLOPs on device.
"""

import numpy as np
import ml_dtypes

import concourse.bass as bass
import concourse.bacc as bacc
import concourse.tile as tile
from concourse import mybir
from concourse.bass_utils import run_bass_kernel_spmd

BF16 = mybir.dt.bfloat16
F32 = mybir.dt.float32

B, N, C = 8, 16384, 64
H, DH, SR, NK = 2, 32, 8, 256
EPS = 1e-5
NCORES = 8

_CACHE = {}


def _bf16(a):
    return np.asarray(a, dtype=ml_dtypes.bfloat16)


def build_graph(reps=1):
    nc = bacc.Bacc("TRN2", target_bir_lowering=False)

    xt_d = nc.declare_dram_parameter("xt", [C, N], BF16, isOutput=False)
    packb_d = nc.declare_dram_parameter("packb", [128, 5120], BF16,
                                        isOutput=False)
    out_d = nc.declare_dram_parameter("out", [N, C], F32, isOutput=True)

    with tile.TileContext(nc) as tc:
        with (
            tc.tile_pool(name="persist", bufs=1) as pp,
            tc.tile_pool(name="work", bufs=3) as wp,
        ):
          packb_sb = pp.tile([128, 5120], BF16, tag="packb")
          nc.sync.dma_start(out=packb_sb[:], in_=packb_d[:])
          for rep in range(reps):
            # ---- load persistent tensors (3 DMAs total) ----
            xt_sb = pp.tile([C, N], BF16, tag="xt")
            nc.sync.dma_start(out=xt_sb[:], in_=xt_d[:])
            wqd_sb = packb_sb[0:64, 0:128]
            wsr_sb = packb_sb[0:64, 128:4224].rearrange(
                "c (p o) -> c p o", p=64)
            wkvg_sb = packb_sb[0:64, 4224:4352]
            ncs_sb = packb_sb[0:1, 4352:4480]
            inv64_sb = packb_sb[0:64, 4480:4481]
            ident_sb = packb_sb[:, 4481:4513]
            wprojA_sb = packb_sb[0:32, 4513:4577]
            wprojB_sb = packb_sb[0:32, 5025:5089]
            wprojAB = [wprojA_sb, wprojB_sb]
            one32_sb = packb_sb[32:33, 5089:5090]
            bsrrow_sb = packb_sb[0:1, 4577:4641]
            ckvrow_sb = packb_sb[0:1, 4641:4769]
            onesrow_sb = packb_sb[0:1, 4769:5025]

            qt_sb = pp.tile([128, N], BF16, tag="qt")
            kvt_sb = pp.tile([128, NK], BF16, tag="kvt")
            krep_sb = pp.tile([128, NK], BF16, tag="krep")
            vext = [
                [pp.tile([128, 64], BF16, tag=f"vext{h}{c}",
                         name=f"vext{h}{c}_{rep}") for c in range(2)]
                for h in range(2)
            ]

            # ================= prelude: conv + LN stats + kv =================
            with tc.tile_pool(name="pre_ps", bufs=1, space="PSUM") as pps:
                conv_ps = pps.tile([C, NK], F32, tag="conv")
                xv = xt_sb[:].rearrange("c (ph i pw j) -> c ph i pw j",
                                        ph=16, i=8, pw=16, j=8)
                for pos in range(64):
                    i, j = pos // 8, pos % 8
                    nc.tensor.matmul(
                        conv_ps[:], wsr_sb[:, pos, :], xv[:, :, i, :, j],
                        start=(pos == 0), stop=False,
                    )
                nc.tensor.matmul(conv_ps[:], bsrrow_sb, onesrow_sb,
                                 start=False, stop=True)
                c_sb = pp.tile([C, NK], BF16, tag="c_sb")
                nc.vector.tensor_copy(c_sb[:], conv_ps[:])
                sq_sb = pp.tile([C, NK], BF16, tag="sq_sb")
                nc.scalar.square(sq_sb[:], c_sb[:])

                mu_ps = pps.tile([1, NK], F32, tag="mu")
                nc.tensor.matmul(mu_ps[:], inv64_sb, c_sb[:],
                                 start=True, stop=True)
                m2_ps = pps.tile([1, NK], F32, tag="m2")
                nc.tensor.matmul(m2_ps[:], inv64_sb, sq_sb[:],
                                 start=True, stop=True)

                mu16_sb = pp.tile([1, NK], BF16, tag="mu16")
                nc.vector.tensor_copy(mu16_sb[:], mu_ps[:])
                mu2_sb = wp.tile([1, NK], F32, tag="mu2")
                nc.vector.tensor_tensor(mu2_sb[:], mu16_sb[:], mu16_sb[:],
                                        mybir.AluOpType.mult)
                var_sb = wp.tile([1, NK], F32, tag="var")
                nc.vector.tensor_tensor(var_sb[:], m2_ps[:], mu2_sb[:],
                                        mybir.AluOpType.subtract)
                veps_sb = wp.tile([1, NK], F32, tag="veps")
                nc.vector.tensor_scalar_add(veps_sb[:], var_sb[:], EPS)
                rvar_sb = wp.tile([1, NK], F32, tag="rvar")
                nc.vector.reciprocal(rvar_sb[:], veps_sb[:])
                rstd_sb = pp.tile([1, NK], F32, tag="rstd")
                nc.scalar.sqrt(rstd_sb[:], rvar_sb[:])

                # kvT = WkvgT @ (conv+bsr) - colsum x mu, then *rstd + ckv
                kv_ps = pps.tile([128, NK], F32, tag="kv")
                nc.tensor.matmul(kv_ps[:], wkvg_sb, c_sb[:],
                                 start=True, stop=False)
                nc.tensor.matmul(kv_ps[:], ncs_sb, mu16_sb[:],
                                 start=False, stop=False)
                nc.tensor.matmul(kv_ps[:], ckvrow_sb, onesrow_sb,
                                 start=False, stop=True)
                rstd_rep = pp.tile([128, NK], F32, tag="rstd_rep",
                                   name=f"rstd_rep{rep}")
                nc.gpsimd.partition_broadcast(rstd_rep[:], rstd_sb[:])
                nc.vector.tensor_tensor(
                    kvt_sb[:], kv_ps[:], rstd_rep[:],
                    mybir.AluOpType.mult)

                # krep: kT_h0 at rows 0-31, kT_h1 at rows 64-95
                nc.vector.tensor_copy(krep_sb[0:32, :], kvt_sb[0:32, :])
                nc.sync.dma_start(out=krep_sb[64:96, :], in_=kvt_sb[32:64, :])

                # v natural layout per (head, key-chunk) + ones column
                for h in range(2):
                    for ch in range(2):
                        tp_ps = pps.tile([128, 32], BF16, tag="tp")
                        nc.tensor.matmul(
                            tp_ps[:],
                            kvt_sb[64 + 32 * h:96 + 32 * h,
                                   128 * ch:128 * (ch + 1)],
                            ident_sb[64 + 32 * h:96 + 32 * h, :],
                            is_transpose=True,
                            tile_position=(64 + 32 * h, 0),
                            start=True, stop=True)
                        nc.vector.memset(vext[h][ch][:, 32:64], 0.0)
                        nc.vector.memset(vext[h][ch][:, 32:33], 1.0)
                        nc.vector.tensor_copy(vext[h][ch][:, 0:32], tp_ps[:])

                # ---- qT (dup layout rows = [qh0, qh1, qh0, qh1]) ----
                for t in range(32):
                    qt_ps = pps.tile([128, 512], F32, tag="qtp")
                    nc.tensor.matmul(
                        qt_ps[:], wqd_sb, xt_sb[:, 512 * t:512 * (t + 1)],
                        start=True, stop=True)
                    nc.vector.tensor_copy(
                        qt_sb[:, 512 * t:512 * (t + 1)], qt_ps[:])

            # ================= attention main loop =================
            TCH = 256          # tokens per chunk
            NCH = N // TCH     # 64 chunks
            GRP = 4            # chunks per epilogue group
            with (
                tc.tile_pool(name="st_ps", bufs=2, space="PSUM") as stp,
                tc.tile_pool(name="av_ps", bufs=1, space="PSUM") as avp,
                tc.tile_pool(name="y_ps", bufs=2, space="PSUM") as yp,
                tc.tile_pool(name="rd_ps", bufs=1, space="PSUM") as rdp,
                tc.tile_pool(name="att_sb", bufs=4) as asb,
                tc.tile_pool(name="grp_sb", bufs=2) as gsb,
            ):
                for g in range(NCH // GRP):
                    # av_sb: [64, chunk, 512] ; per chunk h0 toks at
                    # [0:256], h1 at [256:512]; row 32 = denominator
                    av_sb = gsb.tile([C, GRP, 512], BF16, tag="av_sb")
                    for cc in range(GRP):
                        t = g * GRP + cc
                        st_ps = stp.tile([128, 4 * TCH], F32, tag="st")
                        for hh in range(2):
                            for kc in range(2):
                                qr = 2 * hh + kc
                                rg = 64 * hh
                                nc.tensor.matmul(
                                    st_ps[:, TCH * qr:TCH * (qr + 1)],
                                    krep_sb[rg:rg + 32,
                                            128 * kc:128 * (kc + 1)],
                                    qt_sb[rg:rg + 32,
                                          TCH * t:TCH * (t + 1)],
                                    start=True, stop=True,
                                    tile_position=(rg, 0),
                                    skip_group_check=True,
                                )
                        est_sb = asb.tile([128, 4 * TCH], BF16, tag="est")
                        nc.scalar.activation(est_sb[:], st_ps[:],
                                             mybir.ActivationFunctionType.Exp)
                        av_ps = avp.tile([C, 512], F32, tag="av")
                        for hh in range(2):
                            for kc in range(2):
                                nc.tensor.matmul(
                                    av_ps[:, 256 * hh:256 * (hh + 1)],
                                    vext[hh][kc][:],
                                    est_sb[:, TCH * (2 * hh + kc):
                                           TCH * (2 * hh + kc + 1)],
                                    start=(kc == 0), stop=(kc == 1),
                                    skip_group_check=True,
                                )
                        nc.vector.tensor_copy(av_sb[:, cc, :], av_ps[:])

                    # denominator rows -> columns via K=1 matmuls
                    rd_ps = rdp.tile([128, 16], F32, tag="rd")
                    for sl in range(2 * GRP):
                        chunk, wi = sl // 2, sl % 2
                        for hh in range(2):
                            nc.tensor.matmul(
                                rd_ps[:, 8 * hh + sl:8 * hh + sl + 1],
                                av_sb[32:33, chunk,
                                      256 * hh + 128 * wi:
                                      256 * hh + 128 * wi + 128],
                                one32_sb,
                                start=True, stop=True,
                                tile_position=(32, 0),
                                skip_group_check=True,
                            )
                    rdinv_sb = gsb.tile([128, 16], F32, tag="rdinv")
                    nc.vector.reciprocal(rdinv_sb[:], rd_ps[:])

                    y_sb = gsb.tile([128, GRP * 2, C], F32, tag="y_sb")
                    for sl in range(GRP * 2):
                        chunk, wi = sl // 2, sl % 2
                        y2_ps = yp.tile([128, 2 * C], F32, tag="y")
                        for hh in range(2):
                            nc.tensor.matmul(
                                y2_ps[:, C * hh:C * (hh + 1)],
                                av_sb[0:32, chunk,
                                      256 * hh + 128 * wi:
                                      256 * hh + 128 * wi + 128],
                                wprojAB[hh],
                                start=True, stop=True,
                                skip_group_check=True,
                            )
                        tA_sb = wp.tile([128, C], F32, tag="tA")
                        nc.vector.tensor_scalar_mul(
                            tA_sb[:], y2_ps[:, 0:C],
                            rdinv_sb[:, sl:sl + 1])
                        tB_sb = wp.tile([128, C], F32, tag="tB")
                        nc.vector.tensor_scalar_mul(
                            tB_sb[:], y2_ps[:, C:2 * C],
                            rdinv_sb[:, 8 + sl:8 + sl + 1])
                        nc.vector.tensor_tensor(
                            y_sb[:, sl, :], tA_sb[:], tB_sb[:],
                            mybir.AluOpType.add)
                    ov = out_d[:].rearrange("(g s p) c -> g p s c",
                                            g=NCH // GRP, s=GRP * 2, p=128)
                    nc.sync.dma_start(out=ov[g], in_=y_sb[:])

    nc.compile()
    return nc


def _prep_inputs(x, height, width, Wq, Wkv, Wsr, b_sr, ln_g, ln_b,
                 Wproj, b_proj):
    x = np.asarray(x, np.float32)
    Wq = np.asarray(Wq, np.float32)
    Wkv = np.asarray(Wkv, np.float32)
    Wsr = np.asarray(Wsr, np.float32)
    b_sr = np.asarray(b_sr, np.float32)
    ln_g = np.asarray(ln_g, np.float32)
    ln_b = np.asarray(ln_b, np.float32)
    Wproj = np.asarray(Wproj, np.float32)

    scale = float(DH) ** -0.5
    wq_s = Wq * scale
    wqd = np.zeros((C, 128), np.float32)
    wqd[:, 0:32] = wq_s[:, 0:32]    # head0 -> psum rows 0-31
    wqd[:, 64:96] = wq_s[:, 32:64]  # head1 -> psum rows 64-95
    wsr = Wsr.transpose(1, 2, 3, 0).reshape(C, 64 * C)          # [ci,pos*co]
    wkvg = ln_g[:, None] * Wkv                                  # [64,128]
    ncs = (-wkvg.sum(axis=0)).reshape(1, 128)
    ckv = (ln_b @ Wkv).reshape(128, 1).astype(np.float32)
    packb = np.zeros((128, 5120), np.float32)
    packb[0:64, 0:128] = wqd
    packb[0:64, 128:4224] = wsr
    packb[0:64, 4224:4352] = wkvg
    packb[0:1, 4352:4480] = ncs
    packb[0:64, 4480] = 1.0 / C
    packb[:, 4481:4513] = np.tile(np.eye(32, dtype=np.float32), (4, 1))
    packb[0:32, 4513:4577] = Wproj[0:32]
    packb[0:32, 5025:5089] = Wproj[32:64]
    packb[0, 4577:4641] = b_sr
    packb[0, 4641:4769] = ckv[:, 0]
    packb[0, 4769:5025] = 1.0
    packb[32, 5089] = 1.0
    packb = _bf16(packb)
    shared = dict(packb=packb)
    in_maps = []
    for b in range(B):
        m = dict(shared)
        m["xt"] = _bf16(np.ascontiguousarray(x[b].T))
        in_maps.append(m)
    return in_maps


def kernel(x, height, width, Wq, Wkv, Wsr, b_sr, ln_g, ln_b, Wproj, b_proj,
           _want_time=False):
    assert int(height) == 128 and int(width) == 128
    in_maps = _prep_inputs(x, height, width, Wq, Wkv, Wsr, b_sr, ln_g, ln_b,
                           Wproj, b_proj)
    if "nc" not in _CACHE:
        _CACHE["nc"] = build_graph()
    nc = _CACHE["nc"]
    import os
    trace = bool(int(os.environ.get("BASS_KERNEL_TRACE", "0")))
    res = run_bass_kernel_spmd(nc, in_maps, core_ids=list(range(NCORES)),
                               trace=trace)
    outs = [np.asarray(res.results[i]["out"], np.float32) for i in range(B)]
    out = np.stack(outs, axis=0)
    out = out + np.asarray(b_proj, np.float32)[None, None, :]
    if _want_time:
        return out, res
    return out

